# revision 6
# baseline (speedup 1.0000x reference)
"""Trainium2 Bass kernel for nn_Discriminator (2-layer Keras GRU + dense).

Self-contained: builds per-core one-hot/table inputs on host, runs a
units-on-partition GRU scan on 8 NeuronCores (data-parallel over batch),
returns [512, 1] fp32 logits.

Math (Keras GRU v2, reset_after=True, mask_zero=True), per layer:
    z = sigmoid(xz + rz);  r = sigmoid(xr + rr)
    hh = tanh(xh + r * rh);  h' = m ? (z*h + (1-z)*hh) : h
Rewritten with zbar = 1 - z = sigmoid(-(xz + rz)):
    h' = u - v,  u = zbar * hh,  v = (zbar - 1) * h
Masking: token==0 rows of the layer-1 zbar table (and a dedicated layer-2
mask matmul) add -BIG to the zbar preactivation, forcing zbar ~= 0, i.e.
h' = h exactly.

Input projections: x = emb[tokens] collapses to table lookups of
EW = emb @ W, implemented on-device as matmuls against one-hot columns
(K=26 incl. a ones-row that carries biases).
"""

import json
import os
import sys

import numpy as np

sys.path.insert(0, "/opt/trn_rl_repo")

B, T, V, E, U = 512, 512, 25, 128, 100
NCORES = 8
BL = B // NCORES  # 64 batch per core
BIG = 40.0
NB = 8  # h-state rotation buffers (columns of h1buf/h2buf)

_NOP_ID = [0]


def _split_waits(bir_json_bytes: bytes, cap: int = 1) -> bytes:
    """This walrus build accepts at most `cap` sem waits per instruction.
    Hoist excess waits onto preceding same-engine Drain clones."""
    j = json.loads(bir_json_bytes)
    for fn in j["functions"]:
        for blk in fn["blocks"]:
            ins_list = blk.get("instructions")
            if not ins_list:
                continue
            out = []
            for ins in ins_list:
                si = ins.get("sync_info") or {}
                waits = si.get("on_wait") or []
                if len(waits) > cap:
                    extra = waits[: len(waits) - cap]
                    keep = waits[len(waits) - cap:]
                    for i in range(0, len(extra), cap):
                        _NOP_ID[0] += 1
                        out.append({
                            "name": f"I-waitfix-{_NOP_ID[0]}",
                            "opcode": "EventSemaphore",
                            "engine": ins["engine"],
                            "debug": ins.get("debug", 0),
                            "ins": [],
                            "outs": [],
                            "sync_info": {
                                "on_update": [],
                                "on_wait": extra[i : i + cap],
                            },
                        })
                    si["on_wait"] = keep
                out.append(ins)
            blk["instructions"] = out
    return json.dumps(j).encode()


def _install_compile_patch():
    import concourse.bass_utils as bass_utils
    import concourse.bass2jax as bass2jax

    if getattr(bass_utils, "_ant_waitfix_installed", False):
        return
    orig = bass_utils.compile_bir_kernel

    def patched(bir_json, tmpdir, neff_name="file.neff"):
        return orig(_split_waits(bir_json), tmpdir, neff_name)

    bass_utils.compile_bir_kernel = patched
    bass2jax.compile_bir_kernel = patched
    bass_utils._ant_waitfix_installed = True


def _build_tables(emb, W1, U1, b1, W2, U2, b2):
    """Host-side weight fusion -> stationary lhsT tables (fp32)."""
    f32 = np.float32
    EW1 = (emb.astype(np.float64) @ W1.astype(np.float64)).astype(f32)  # [25,300]
    b1sum = (b1[0] + b1[1]).astype(f32)
    b2sum = (b2[0] + b2[1]).astype(f32)

    def rep4(base26):  # [26,100] -> [128,100] replicated at partition 0/32/64/96
        t = np.zeros((128, U), f32)
        for k in range(4):
            t[32 * k : 32 * k + 26, :] = base26
        return t

    az = np.zeros((26, U), f32)
    az[:25] = -EW1[:, 0:U]
    az[25] = -b1sum[0:U]
    az[0] -= BIG  # token 0 => masked => zbar ~= 0
    ar = np.zeros((26, U), f32)
    ar[:25] = EW1[:, U : 2 * U]
    ar[25] = b1sum[U : 2 * U]
    ax = np.zeros((26, U), f32)
    ax[:25] = EW1[:, 2 * U : 3 * U]
    ax[25] = b1[0, 2 * U : 3 * U]
    am = np.zeros((26, U), f32)
    am[0] = -BIG

    def aug(mat, biasrow):  # [100,100] + bias row -> [101,100]
        t = np.zeros((101, U), f32)
        t[:100] = mat
        t[100] = biasrow
        return t

    z = np.zeros(U, f32)
    tables = {
        "azb": rep4(az), "ar": rep4(ar), "axh": rep4(ax), "am2": rep4(am),
        "uzb": aug(-U1[:, 0:U], z),
        "ur": aug(U1[:, U : 2 * U], z),
        "urh": aug(U1[:, 2 * U :], b1[1, 2 * U :]),
        "wzb2": aug(-W2[:, 0:U], -b2sum[0:U]),
        "wr2": aug(W2[:, U : 2 * U], b2sum[U : 2 * U]),
        "wxh2": aug(W2[:, 2 * U :], b2[0, 2 * U :]),
        "uzb2": aug(-U2[:, 0:U], z),
        "ur2": aug(U2[:, U : 2 * U], z),
        "urh2": aug(U2[:, 2 * U :], b2[1, 2 * U :]),
    }
    return {k: np.ascontiguousarray(v) for k, v in tables.items()}


def _build_onehot(tok_core):
    """tok_core [BL, T] int32 -> packed one-hot [128, (T//4)*BL] fp32.

    Step t lives at partition base (t%4)*32 rows 0..25 (row 25 = ones),
    free cols (t//4)*BL .. +BL.
    """
    oh = np.zeros((128, (T // 4) * BL), np.float32)
    for t in range(T):
        pb = (t % 4) * 32
        fc = (t // 4) * BL
        cols = tok_core[:, t]  # [BL]
        oh[pb + cols, fc + np.arange(BL)] = 1.0
        oh[pb + 25, fc : fc + BL] = 1.0
    return oh


def _build_program(nsteps=T):
    import concourse.bass as bass
    import concourse.tile as tile
    from concourse import mybir

    AFT = mybir.ActivationFunctionType
    Alu = mybir.AluOpType
    f32 = mybir.dt.float32

    nc = bass.Bass()
    OHW = (T // 4) * BL  # 8192
    d_oh = nc.dram_tensor("oh", [128, OHW], f32, kind="ExternalInput")
    d_tabs = {}
    for n in ("azb", "ar", "axh", "am2"):
        d_tabs[n] = nc.dram_tensor(n, [128, U], f32, kind="ExternalInput")
    for n in ("uzb", "ur", "urh", "wzb2", "wr2", "wxh2", "uzb2", "ur2", "urh2"):
        d_tabs[n] = nc.dram_tensor(n, [101, U], f32, kind="ExternalInput")
    d_wd = nc.dram_tensor("wd", [U, 1], f32, kind="ExternalInput")
    d_bd = nc.dram_tensor("bd", [BL, 1], f32, kind="ExternalInput")
    d_ones = nc.dram_tensor("ones", [1, NB * BL], f32, kind="ExternalInput")
    d_y = nc.dram_tensor("y", [BL, 1], f32, kind="ExternalOutput")

    with tile.TileContext(nc) as tc:
        with (
            tc.tile_pool(name="const", bufs=1) as cp,
            tc.tile_pool(name="ps1", bufs=3, space="PSUM") as ps1,
            tc.tile_pool(name="ps2", bufs=3, space="PSUM") as ps2,
            tc.tile_pool(name="psd", bufs=1, space="PSUM") as psd,
            tc.tile_pool(name="gates", bufs=6) as gp,
        ):
            # ---- constants / state buffers ----
            oh_sb = cp.tile([128, OHW], f32, tag="oh")
            for c in range(8):
                sl = slice(c * OHW // 8, (c + 1) * OHW // 8)
                nc.sync.dma_start(oh_sb[:, sl], d_oh[:, sl])
            tabs = {}
            for n, d in d_tabs.items():
                tabs[n] = cp.tile(list(d.shape), f32, tag=n, name=n)
                nc.sync.dma_start(tabs[n][:], d[:])
            wd_sb = cp.tile([U, 1], f32, tag="wd")
            nc.sync.dma_start(wd_sb[:], d_wd[:])
            bd_sb = cp.tile([BL, 1], f32, tag="bd")
            nc.sync.dma_start(bd_sb[:], d_bd[:])
            h1buf = cp.tile([101, NB * BL], f32, tag="h1buf")
            h2buf = cp.tile([101, NB * BL], f32, tag="h2buf")
            nc.vector.memset(h1buf[:], 0.0)
            nc.vector.memset(h2buf[:], 0.0)
            nc.sync.dma_start(h1buf[100:101, :], d_ones[:])
            nc.sync.dma_start(h2buf[100:101, :], d_ones[:])
            out_sb = cp.tile([BL, 1], f32, tag="out")

            def gate_block(P, hprev_state, hbuf_out_cols):
                """Common per-layer gate math. P: psum [100,256] with
                blocks zbar|r|xh|rech. Writes h' into hbuf cols."""
                s = gp.tile([U, 2 * BL], f32, tag="s")
                nc.scalar.activation(s[:], P[:, 0 : 2 * BL], AFT.Sigmoid)
                q = gp.tile([U, BL], f32, tag="q")
                nc.vector.tensor_tensor(
                    q[:], s[:, BL : 2 * BL], P[:, 3 * BL : 4 * BL], op=Alu.mult
                )
                p = gp.tile([U, BL], f32, tag="p")
                nc.vector.tensor_tensor(
                    p[:], q[:], P[:, 2 * BL : 3 * BL], op=Alu.add
                )
                hh = gp.tile([U, BL], f32, tag="hh")
                nc.scalar.activation(hh[:], p[:], AFT.Tanh)
                v = gp.tile([U, BL], f32, tag="v")
                nc.vector.scalar_tensor_tensor(
                    v[:], s[:, 0:BL], 1.0, hprev_state,
                    op0=Alu.subtract, op1=Alu.mult,
                )
                u = gp.tile([U, BL], f32, tag="u")
                nc.gpsimd.tensor_tensor(u[:], s[:, 0:BL], hh[:], op=Alu.mult)
                nc.vector.tensor_tensor(hbuf_out_cols, u[:], v[:], op=Alu.subtract)

            mm = nc.tensor.matmul
            for t in range(nsteps):
                pb = (t % 4) * 32
                fc = (t // 4) * BL
                oh = oh_sb[pb : pb + 26, fc : fc + BL]
                tp = (pb, 0)
                c_t = (t % NB) * BL
                c_p = ((t - 1) % NB) * BL
                h1_prev = h1buf[:, c_p : c_p + BL]          # [101, BL]
                h1_prev_s = h1buf[0:100, c_p : c_p + BL]    # state rows only
                h2_prev = h2buf[:, c_p : c_p + BL]
                h2_prev_s = h2buf[0:100, c_p : c_p + BL]

                # ---- layer 1, step t ----
                P1 = ps1.tile([U, 4 * BL], f32, tag="P1")
                # NOTE: start=True clears has_written for the WHOLE bank, so
                # only the first matmul per PSUM tile may carry it.
                mm(P1[:, 0:BL], tabs["azb"][pb : pb + 26, :], oh,
                   start=True, stop=False, tile_position=tp, skip_group_check=True)
                mm(P1[:, BL : 2 * BL], tabs["ar"][pb : pb + 26, :], oh,
                   start=False, stop=False, tile_position=tp, skip_group_check=True)
                mm(P1[:, 2 * BL : 3 * BL], tabs["axh"][pb : pb + 26, :], oh,
                   start=False, stop=False, tile_position=tp, skip_group_check=True)
                mm(P1[:, 0:BL], tabs["uzb"][:], h1_prev,
                   start=False, stop=False, skip_group_check=True)
                mm(P1[:, BL : 2 * BL], tabs["ur"][:], h1_prev,
                   start=False, stop=False, skip_group_check=True)
                mm(P1[:, 3 * BL : 4 * BL], tabs["urh"][:], h1_prev,
                   start=False, stop=True, skip_group_check=True)
                gate_block(P1, h1_prev_s, h1buf[0:100, c_t : c_t + BL])

                h1_cur = h1buf[:, c_t : c_t + BL]
                # ---- layer 2, step t ----
                P2 = ps2.tile([U, 4 * BL], f32, tag="P2")
                mm(P2[:, 0:BL], tabs["am2"][pb : pb + 26, :], oh,
                   start=True, stop=False, tile_position=tp, skip_group_check=True)
                mm(P2[:, 0:BL], tabs["uzb2"][:], h2_prev,
                   start=False, stop=False, skip_group_check=True)
                mm(P2[:, BL : 2 * BL], tabs["ur2"][:], h2_prev,
                   start=False, stop=False, skip_group_check=True)
                mm(P2[:, 3 * BL : 4 * BL], tabs["urh2"][:], h2_prev,
                   start=False, stop=False, skip_group_check=True)
                mm(P2[:, 0:BL], tabs["wzb2"][:], h1_cur,
                   start=False, stop=False, skip_group_check=True)
                mm(P2[:, BL : 2 * BL], tabs["wr2"][:], h1_cur,
                   start=False, stop=False, skip_group_check=True)
                mm(P2[:, 2 * BL : 3 * BL], tabs["wxh2"][:], h1_cur,
                   start=False, stop=True, skip_group_check=True)
                gate_block(P2, h2_prev_s, h2buf[0:100, c_t : c_t + BL])

            # ---- dense head ----
            c_last = ((nsteps - 1) % NB) * BL
            pd = psd.tile([BL, 1], f32, tag="pd")
            mm(pd[:], h2buf[0:100, c_last : c_last + BL], wd_sb[:],
               start=True, stop=True)
            nc.scalar.activation(out_sb[:], pd[:], AFT.Identity,
                                 bias=bd_sb[:, 0:1])
            nc.sync.dma_start(d_y[:], out_sb[:])

    return nc


_CACHE = {}


def kernel(tokens, emb, W1, U1, b1, W2, U2, b2, Wd, bd):
    _install_compile_patch()
    from concourse.bass_utils import run_bass_kernel_spmd

    tokens = np.asarray(tokens)
    tables = _build_tables(
        np.asarray(emb, np.float32), np.asarray(W1, np.float32),
        np.asarray(U1, np.float32), np.asarray(b1, np.float32),
        np.asarray(W2, np.float32), np.asarray(U2, np.float32),
        np.asarray(b2, np.float32),
    )
    wd = np.ascontiguousarray(np.asarray(Wd, np.float32).reshape(U, 1))
    bdv = np.full((BL, 1), np.float32(np.asarray(bd).reshape(-1)[0]), np.float32)
    ones = np.ones((1, NB * BL), np.float32)

    if "nc" not in _CACHE:
        _CACHE["nc"] = _build_program()
    nc = _CACHE["nc"]

    in_maps = []
    for c in range(NCORES):
        m = {"oh": _build_onehot(tokens[c * BL : (c + 1) * BL])}
        m.update(tables)
        m["wd"] = wd
        m["bd"] = bdv
        m["ones"] = ones
        in_maps.append(m)

    res = run_bass_kernel_spmd(nc, in_maps, core_ids=list(range(NCORES)))
    _CACHE["last_result"] = res
    out = np.concatenate([res.results[c]["y"] for c in range(NCORES)], axis=0)
    return out.astype(np.float32)


if __name__ == "__main__":
    tok = np.random.randint(0, V, (B, T), dtype=np.int32)
    rng = np.random.default_rng(0)
    args = dict(
        tokens=tok,
        emb=rng.normal(size=(V, E)).astype(np.float32) * 0.05,
        W1=rng.normal(size=(E, 3 * U)).astype(np.float32) * 0.08,
        U1=rng.normal(size=(U, 3 * U)).astype(np.float32) * 0.1,
        b1=np.zeros((2, 3 * U), np.float32),
        W2=rng.normal(size=(U, 3 * U)).astype(np.float32) * 0.1,
        U2=rng.normal(size=(U, 3 * U)).astype(np.float32) * 0.1,
        b2=np.zeros((2, 3 * U), np.float32),
        Wd=rng.normal(size=(U, 1)).astype(np.float32) * 0.1,
        bd=np.zeros((1,), np.float32),
    )
    print(kernel(**args)[:4])


# revision 8
# speedup vs baseline: 1.1504x; 1.1504x over previous
"""Trainium2 Bass kernel for nn_Discriminator (2-layer Keras GRU + dense).

Self-contained: builds per-core one-hot/table inputs on host, runs a
units-on-partition GRU scan on 8 NeuronCores (data-parallel over batch),
returns [512, 1] fp32 logits.

Math (Keras GRU v2, reset_after=True, mask_zero=True), per layer:
    z = sigmoid(xz + rz);  r = sigmoid(xr + rr)
    hh = tanh(xh + r * rh);  h' = m ? (z*h + (1-z)*hh) : h
Rewritten with zbar = 1 - z = sigmoid(-(xz + rz)):
    h' = u - v,  u = zbar * hh,  v = (zbar - 1) * h
Masking: token==0 rows of the layer-1 zbar table (and a dedicated layer-2
mask matmul) add -BIG to the zbar preactivation, forcing zbar ~= 0, i.e.
h' = h exactly.

Input projections: x = emb[tokens] collapses to table lookups of
EW = emb @ W, implemented on-device as matmuls against one-hot columns
(K=26 incl. a ones-row that carries biases).
"""

import json
import os
import sys

import numpy as np

sys.path.insert(0, "/opt/trn_rl_repo")

B, T, V, E, U = 512, 512, 25, 128, 100
NCORES = 8
BL = B // NCORES  # 64 batch per core
BIG = 40.0
NB = 8  # stacked-tile rotation depth
MM_DTYPE = "f32"  # f32 | f32r | bf16

_NOP_ID = [0]


def _split_waits(bir_json_bytes: bytes, cap: int = 1) -> bytes:
    """This walrus build accepts at most `cap` sem waits per instruction.
    Hoist excess waits onto preceding same-engine Drain clones."""
    j = json.loads(bir_json_bytes)
    for fn in j["functions"]:
        for blk in fn["blocks"]:
            ins_list = blk.get("instructions")
            if not ins_list:
                continue
            out = []
            for ins in ins_list:
                si = ins.get("sync_info") or {}
                waits = si.get("on_wait") or []
                if len(waits) > cap:
                    extra = waits[: len(waits) - cap]
                    keep = waits[len(waits) - cap:]
                    for i in range(0, len(extra), cap):
                        _NOP_ID[0] += 1
                        out.append({
                            "name": f"I-waitfix-{_NOP_ID[0]}",
                            "opcode": "EventSemaphore",
                            "engine": ins["engine"],
                            "debug": ins.get("debug", 0),
                            "ins": [],
                            "outs": [],
                            "sync_info": {
                                "on_update": [],
                                "on_wait": extra[i : i + cap],
                            },
                        })
                    si["on_wait"] = keep
                out.append(ins)
            blk["instructions"] = out
    return json.dumps(j).encode()


def _install_compile_patch():
    import concourse.bass_utils as bass_utils
    import concourse.bass2jax as bass2jax

    if getattr(bass_utils, "_ant_waitfix_installed", False):
        return
    orig = bass_utils.compile_bir_kernel

    def patched(bir_json, tmpdir, neff_name="file.neff"):
        return orig(_split_waits(bir_json), tmpdir, neff_name)

    bass_utils.compile_bir_kernel = patched
    bass2jax.compile_bir_kernel = patched
    bass_utils._ant_waitfix_installed = True


def _build_tables(emb, W1, U1, b1, W2, U2, b2, np_dt=np.float32):
    """Host-side weight fusion -> stationary lhsT tables.

    Stacked-rhs layout (K=127): rows 0-99 = h, row 100 = ones,
    rows 101-125 = one-hot(token), row 126 = oh ones-row (unused).
    """
    f32 = np.float32
    EW1 = (emb.astype(np.float64) @ W1.astype(np.float64)).astype(f32)  # [25,300]
    b1sum = (b1[0] + b1[1]).astype(f32)
    b2sum = (b2[0] + b2[1]).astype(f32)

    def stk(hpart, biasrow, ohpart):  # -> [127, 100]
        t = np.zeros((127, U), f32)
        if hpart is not None:
            t[:100] = hpart
        t[100] = biasrow
        if ohpart is not None:
            t[101:126] = ohpart
        return t

    def aug(mat, biasrow):  # [100,100] + bias row -> [101,100]
        t = np.zeros((101, U), f32)
        t[:100] = mat
        t[100] = biasrow
        return t

    z = np.zeros(U, f32)
    t1z = stk(-U1[:, 0:U], -b1sum[0:U], -EW1[:, 0:U])
    t1z[101] -= BIG  # token 0 => masked => zbar ~= 0
    t2z = stk(-U2[:, 0:U], -b2sum[0:U], None)
    t2z[101] = -BIG  # layer-2 mask rides the oh part of stk2
    tables = {
        "t1z": t1z,
        "t1r": stk(U1[:, U : 2 * U], b1sum[U : 2 * U], EW1[:, U : 2 * U]),
        "t1x": stk(None, b1[0, 2 * U :], EW1[:, 2 * U :]),
        "t1c": stk(U1[:, 2 * U :], b1[1, 2 * U :], None),
        "t2z": t2z,
        "t2r": stk(U2[:, U : 2 * U], b2sum[U : 2 * U], None),
        "t2c": stk(U2[:, 2 * U :], b2[1, 2 * U :], None),
        "w2z": aug(-W2[:, 0:U], z),
        "w2r": aug(W2[:, U : 2 * U], z),
        "w2x": aug(W2[:, 2 * U :], b2[0, 2 * U :]),
    }
    return {k: np.ascontiguousarray(v.astype(np_dt)) for k, v in tables.items()}


def _build_onehot(tok_core, np_dt=np.float32):
    """tok_core [BL, T] int32 -> one-hot [T, 26, BL] (row 25 = ones),
    DMA'd per step into the stacked rhs tiles at partitions 101..126."""
    oh = np.zeros((T, 26, BL), np_dt)
    ar = np.arange(BL)
    for t in range(T):
        oh[t, tok_core[:, t], ar] = 1.0
        oh[t, 25, :] = 1.0
    return oh


def _build_program(nsteps=T):
    import concourse.bass as bass
    import concourse.tile as tile
    from concourse import mybir

    AFT = mybir.ActivationFunctionType
    Alu = mybir.AluOpType
    f32 = mybir.dt.float32
    mmdt = {"f32": mybir.dt.float32, "f32r": mybir.dt.float32r,
            "bf16": mybir.dt.bfloat16}[MM_DTYPE]
    tdt = f32 if MM_DTYPE in ("f32", "f32r") else mmdt  # tile/dram dtype

    def mmcast(ap):  # reinterpret f32 operands as f32r for 1-pass matmuls
        return ap.bitcast(mybir.dt.float32r) if MM_DTYPE == "f32r" else ap

    nc = bass.Bass()
    d_oh = nc.dram_tensor("oh", [T, 26, BL], tdt, kind="ExternalInput")
    d_tabs = {}
    for n in ("t1z", "t1r", "t1x", "t1c", "t2z", "t2r", "t2c"):
        d_tabs[n] = nc.dram_tensor(n, [127, U], tdt, kind="ExternalInput")
    for n in ("w2z", "w2r", "w2x"):
        d_tabs[n] = nc.dram_tensor(n, [101, U], tdt, kind="ExternalInput")
    d_wd = nc.dram_tensor("wd", [U, 1], tdt, kind="ExternalInput")
    d_bd = nc.dram_tensor("bd", [BL, 1], f32, kind="ExternalInput")
    d_ones = nc.dram_tensor("ones", [1, BL], tdt, kind="ExternalInput")
    d_y = nc.dram_tensor("y", [BL, 1], f32, kind="ExternalOutput")

    with tile.TileContext(nc) as tc:
        with (
            tc.tile_pool(name="const", bufs=1) as cp,
            tc.tile_pool(name="ps1", bufs=3, space="PSUM") as ps1,
            tc.tile_pool(name="ps2", bufs=3, space="PSUM") as ps2,
            tc.tile_pool(name="psd", bufs=1, space="PSUM") as psd,
            tc.tile_pool(name="gates", bufs=6) as gp,
        ):
            tabs = {}
            for n, d in d_tabs.items():
                tabs[n] = cp.tile(list(d.shape), tdt, tag=n, name=n)
                nc.sync.dma_start(tabs[n][:], d[:])
            wd_sb = cp.tile([U, 1], tdt, tag="wd")
            nc.sync.dma_start(wd_sb[:], d_wd[:])
            bd_sb = cp.tile([BL, 1], f32, tag="bd")
            nc.sync.dma_start(bd_sb[:], d_bd[:])
            # stacked rhs state tiles: rows 0-99 h, row 100 ones, 101-126 oh
            stk1 = []
            stk2 = []
            for k in range(NB):
                s1 = cp.tile([127, BL], tdt, tag=f"stk1_{k}", name=f"stk1_{k}")
                s2 = cp.tile([127, BL], tdt, tag=f"stk2_{k}", name=f"stk2_{k}")
                nc.vector.memset(s1[:], 0.0)
                nc.vector.memset(s2[:], 0.0)
                nc.sync.dma_start(s1[100:101, :], d_ones[:])
                nc.sync.dma_start(s2[100:101, :], d_ones[:])
                stk1.append(s1)
                stk2.append(s2)
            out_sb = cp.tile([BL, 1], f32, tag="out")

            def gate_block(P, hprev_state, h_out):
                """P: psum [100, 4*BL] blocks zbar|r|xh|rech -> h' into h_out."""
                s = gp.tile([U, 2 * BL], f32, tag="s")
                nc.scalar.activation(s[:], P[:, 0 : 2 * BL], AFT.Sigmoid)
                q = gp.tile([U, BL], f32, tag="q")
                nc.vector.tensor_tensor(
                    q[:], s[:, BL : 2 * BL], P[:, 3 * BL : 4 * BL], op=Alu.mult
                )
                p = gp.tile([U, BL], f32, tag="p")
                nc.vector.tensor_tensor(
                    p[:], q[:], P[:, 2 * BL : 3 * BL], op=Alu.add
                )
                hh = gp.tile([U, BL], f32, tag="hh")
                nc.scalar.activation(hh[:], p[:], AFT.Tanh)
                v = gp.tile([U, BL], f32, tag="v")
                nc.vector.scalar_tensor_tensor(
                    v[:], s[:, 0:BL], 1.0, hprev_state,
                    op0=Alu.subtract, op1=Alu.mult,
                )
                u = gp.tile([U, BL], f32, tag="u")
                nc.gpsimd.tensor_tensor(u[:], s[:, 0:BL], hh[:], op=Alu.mult)
                nc.vector.tensor_tensor(h_out, u[:], v[:], op=Alu.subtract)

            mm = nc.tensor.matmul
            for t in range(nsteps):
                k_t = t % NB
                k_n = (t + 1) % NB
                s1_t, s1_n = stk1[k_t], stk1[k_n]
                s2_t, s2_n = stk2[k_t], stk2[k_n]

                # one-hot for step t+1 into the next stacked tiles (off-chain)
                if t + 1 < nsteps:
                    nc.sync.dma_start(s1_n[101:127, :], d_oh[t + 1])
                    nc.sync.dma_start(s2_n[101:127, :], d_oh[t + 1])
                if t == 0:
                    nc.sync.dma_start(s1_t[101:127, :], d_oh[0])
                    nc.sync.dma_start(s2_t[101:127, :], d_oh[0])

                # ---- layer 2 recurrent part (only needs h2_{t-1}) ----
                P2 = ps2.tile([U, 4 * BL], f32, tag="P2")
                mm(P2[:, 0:BL], mmcast(tabs["t2z"][:]), mmcast(s2_t[:]),
                   start=True, stop=False, skip_group_check=True)
                mm(P2[:, BL : 2 * BL], mmcast(tabs["t2r"][:]), mmcast(s2_t[:]),
                   start=False, stop=False, skip_group_check=True)
                mm(P2[:, 3 * BL : 4 * BL], mmcast(tabs["t2c"][:]), mmcast(s2_t[:]),
                   start=False, stop=False, skip_group_check=True)

                # ---- layer 1, step t ----
                P1 = ps1.tile([U, 4 * BL], f32, tag="P1")
                mm(P1[:, 0:BL], mmcast(tabs["t1z"][:]), mmcast(s1_t[:]),
                   start=True, stop=False, skip_group_check=True)
                mm(P1[:, BL : 2 * BL], mmcast(tabs["t1r"][:]), mmcast(s1_t[:]),
                   start=False, stop=False, skip_group_check=True)
                mm(P1[:, 2 * BL : 3 * BL], mmcast(tabs["t1x"][:]), mmcast(s1_t[:]),
                   start=False, stop=False, skip_group_check=True)
                mm(P1[:, 3 * BL : 4 * BL], mmcast(tabs["t1c"][:]), mmcast(s1_t[:]),
                   start=False, stop=True, skip_group_check=True)
                gate_block(P1, s1_t[0:100, :], s1_n[0:100, :])

                # ---- layer 2 input part (needs h1_t = s1_n rows 0-100) ----
                mm(P2[:, 0:BL], mmcast(tabs["w2z"][:]), mmcast(s1_n[0:101, :]),
                   start=False, stop=False, skip_group_check=True)
                mm(P2[:, BL : 2 * BL], mmcast(tabs["w2r"][:]), mmcast(s1_n[0:101, :]),
                   start=False, stop=False, skip_group_check=True)
                mm(P2[:, 2 * BL : 3 * BL], mmcast(tabs["w2x"][:]), mmcast(s1_n[0:101, :]),
                   start=False, stop=True, skip_group_check=True)
                gate_block(P2, s2_t[0:100, :], s2_n[0:100, :])

            # ---- dense head ----
            s2_fin = stk2[nsteps % NB]
            pd = psd.tile([BL, 1], f32, tag="pd")
            mm(pd[:], mmcast(s2_fin[0:100, :]), mmcast(wd_sb[:]),
               start=True, stop=True)
            nc.scalar.activation(out_sb[:], pd[:], AFT.Identity,
                                 bias=bd_sb[:, 0:1])
            nc.sync.dma_start(d_y[:], out_sb[:])

    return nc


_CACHE = {}


def kernel(tokens, emb, W1, U1, b1, W2, U2, b2, Wd, bd):
    _install_compile_patch()
    from concourse.bass_utils import run_bass_kernel_spmd

    tokens = np.asarray(tokens)
    np_dt = np.float32 if MM_DTYPE in ("f32", "f32r") else __import__("ml_dtypes").bfloat16
    tables = _build_tables(
        np.asarray(emb, np.float32), np.asarray(W1, np.float32),
        np.asarray(U1, np.float32), np.asarray(b1, np.float32),
        np.asarray(W2, np.float32), np.asarray(U2, np.float32),
        np.asarray(b2, np.float32), np_dt=np_dt,
    )
    wd = np.ascontiguousarray(np.asarray(Wd, np.float32).reshape(U, 1).astype(np_dt))
    bdv = np.full((BL, 1), np.float32(np.asarray(bd).reshape(-1)[0]), np.float32)
    ones = np.ones((1, BL), np_dt)

    if "nc" not in _CACHE:
        _CACHE["nc"] = _build_program()
    nc = _CACHE["nc"]

    in_maps = []
    for c in range(NCORES):
        m = {"oh": _build_onehot(tokens[c * BL : (c + 1) * BL], np_dt)}
        m.update(tables)
        m["wd"] = wd
        m["bd"] = bdv
        m["ones"] = ones
        in_maps.append(m)

    res = run_bass_kernel_spmd(nc, in_maps, core_ids=list(range(NCORES)))
    _CACHE["last_result"] = res
    out = np.concatenate([res.results[c]["y"] for c in range(NCORES)], axis=0)
    return out.astype(np.float32)


if __name__ == "__main__":
    tok = np.random.randint(0, V, (B, T), dtype=np.int32)
    rng = np.random.default_rng(0)
    args = dict(
        tokens=tok,
        emb=rng.normal(size=(V, E)).astype(np.float32) * 0.05,
        W1=rng.normal(size=(E, 3 * U)).astype(np.float32) * 0.08,
        U1=rng.normal(size=(U, 3 * U)).astype(np.float32) * 0.1,
        b1=np.zeros((2, 3 * U), np.float32),
        W2=rng.normal(size=(U, 3 * U)).astype(np.float32) * 0.1,
        U2=rng.normal(size=(U, 3 * U)).astype(np.float32) * 0.1,
        b2=np.zeros((2, 3 * U), np.float32),
        Wd=rng.normal(size=(U, 1)).astype(np.float32) * 0.1,
        bd=np.zeros((1,), np.float32),
    )
    print(kernel(**args)[:4])


# revision 12
# speedup vs baseline: 1.8577x; 1.6147x over previous
"""Trainium2 Bass kernel for nn_Discriminator (2-layer Keras GRU + dense).

Self-contained: builds per-core one-hot/table inputs on host, runs a
units-on-partition GRU scan on 8 NeuronCores (data-parallel over batch),
returns [512, 1] fp32 logits.

Math (Keras GRU v2, reset_after=True, mask_zero=True), per layer:
    z = sigmoid(xz + rz);  r = sigmoid(xr + rr)
    hh = tanh(xh + r * rh);  h' = m ? (z*h + (1-z)*hh) : h
Rewritten with zbar = 1 - z = sigmoid(-(xz + rz)):
    h' = u - v,  u = zbar * hh,  v = (zbar - 1) * h
Masking: token==0 rows of the layer-1 zbar table (and a dedicated layer-2
mask matmul) add -BIG to the zbar preactivation, forcing zbar ~= 0, i.e.
h' = h exactly.

Input projections: x = emb[tokens] collapses to table lookups of
EW = emb @ W, implemented on-device as matmuls against one-hot columns
(K=26 incl. a ones-row that carries biases).
"""

import json
import os
import sys

import numpy as np

sys.path.insert(0, "/opt/trn_rl_repo")

B, T, V, E, U = 512, 512, 25, 128, 100
NCORES = 8
BL = B // NCORES  # 64 batch per core
BIG = 40.0
NB = 8  # stacked-tile rotation depth
MM_DTYPE = "bf16"  # f32 | f32r | bf16

_NOP_ID = [0]


def _split_waits(bir_json_bytes: bytes, cap: int = 1) -> bytes:
    """This walrus build accepts at most `cap` sem waits per instruction.
    Hoist excess waits onto preceding same-engine Drain clones."""
    j = json.loads(bir_json_bytes)
    for fn in j["functions"]:
        for blk in fn["blocks"]:
            ins_list = blk.get("instructions")
            if not ins_list:
                continue
            out = []
            for ins in ins_list:
                si = ins.get("sync_info") or {}
                waits = si.get("on_wait") or []
                if len(waits) > cap:
                    extra = waits[: len(waits) - cap]
                    keep = waits[len(waits) - cap:]
                    for i in range(0, len(extra), cap):
                        _NOP_ID[0] += 1
                        out.append({
                            "name": f"I-waitfix-{_NOP_ID[0]}",
                            "opcode": "EventSemaphore",
                            "engine": ins["engine"],
                            "debug": ins.get("debug", 0),
                            "ins": [],
                            "outs": [],
                            "sync_info": {
                                "on_update": [],
                                "on_wait": extra[i : i + cap],
                            },
                        })
                    si["on_wait"] = keep
                out.append(ins)
            blk["instructions"] = out
    return json.dumps(j).encode()


def _install_compile_patch():
    import concourse.bass_utils as bass_utils
    import concourse.bass2jax as bass2jax

    if getattr(bass_utils, "_ant_waitfix_installed", False):
        return
    orig = bass_utils.compile_bir_kernel

    def patched(bir_json, tmpdir, neff_name="file.neff"):
        return orig(_split_waits(bir_json), tmpdir, neff_name)

    bass_utils.compile_bir_kernel = patched
    bass2jax.compile_bir_kernel = patched
    bass_utils._ant_waitfix_installed = True


def _build_tables(emb, W1, U1, b1, W2, U2, b2, np_dt=np.float32):
    """Host-side weight fusion -> stationary lhsT tables.

    Stacked-rhs layout (K=127): rows 0-99 = h, row 100 = ones,
    rows 101-125 = one-hot(token), row 126 = oh ones-row (unused).
    """
    f32 = np.float32
    EW1 = (emb.astype(np.float64) @ W1.astype(np.float64)).astype(f32)  # [25,300]
    b1sum = (b1[0] + b1[1]).astype(f32)
    b2sum = (b2[0] + b2[1]).astype(f32)

    def stk(hpart, biasrow, ohpart):  # -> [127, 100]
        t = np.zeros((127, U), f32)
        if hpart is not None:
            t[:100] = hpart
        t[100] = biasrow
        if ohpart is not None:
            t[101:126] = ohpart
        return t

    def aug(mat, biasrow):  # [100,100] + bias row -> [101,100]
        t = np.zeros((101, U), f32)
        t[:100] = mat
        t[100] = biasrow
        return t

    z = np.zeros(U, f32)
    t1z = stk(-U1[:, 0:U], -b1sum[0:U], -EW1[:, 0:U])
    t1z[101] -= BIG  # token 0 => masked => zbar ~= 0
    t2z = stk(-U2[:, 0:U], -b2sum[0:U], None)
    t2z[101] = -BIG  # layer-2 mask rides the oh part of stk2
    tables = {
        "t1z": t1z,
        "t1r": stk(U1[:, U : 2 * U], b1sum[U : 2 * U], EW1[:, U : 2 * U]),
        "t1x": stk(None, b1[0, 2 * U :], EW1[:, 2 * U :]),
        "t1c": stk(U1[:, 2 * U :], b1[1, 2 * U :], None),
        "t2z": t2z,
        "t2r": stk(U2[:, U : 2 * U], b2sum[U : 2 * U], None),
        "t2c": stk(U2[:, 2 * U :], b2[1, 2 * U :], None),
        "w2z": aug(-W2[:, 0:U], z),
        "w2r": aug(W2[:, U : 2 * U], z),
        "w2x": aug(W2[:, 2 * U :], b2[0, 2 * U :]),
    }
    return {k: np.ascontiguousarray(v.astype(np_dt)) for k, v in tables.items()}


def _build_onehot(tok_core, np_dt=np.float32):
    """tok_core [BL, T] int32 -> one-hot [T, 26, BL] (row 25 = ones),
    DMA'd per step into the stacked rhs tiles at partitions 101..126."""
    oh = np.zeros((T, 26, BL), np_dt)
    ar = np.arange(BL)
    for t in range(T):
        oh[t, tok_core[:, t], ar] = 1.0
        oh[t, 25, :] = 1.0
    return oh


def _build_program(nsteps=T):
    import concourse.bass as bass
    import concourse.tile as tile
    from concourse import mybir

    AFT = mybir.ActivationFunctionType
    Alu = mybir.AluOpType
    f32 = mybir.dt.float32
    mmdt = {"f32": mybir.dt.float32, "f32r": mybir.dt.float32r,
            "bf16": mybir.dt.bfloat16}[MM_DTYPE]
    tdt = f32 if MM_DTYPE == "f32" else mmdt  # tile/dram dtype (f32r: np f32)

    def mmcast(ap):
        return ap

    nc = bass.Bass()
    d_oh = nc.dram_tensor("oh", [T, 26, BL], tdt, kind="ExternalInput")
    d_tabs = {}
    for n in ("t1z", "t1r", "t1x", "t1c", "t2z", "t2r", "t2c"):
        d_tabs[n] = nc.dram_tensor(n, [127, U], tdt, kind="ExternalInput")
    for n in ("w2z", "w2r", "w2x"):
        d_tabs[n] = nc.dram_tensor(n, [101, U], tdt, kind="ExternalInput")
    d_wd = nc.dram_tensor("wd", [U, 1], tdt, kind="ExternalInput")
    d_bd = nc.dram_tensor("bd", [BL, 1], f32, kind="ExternalInput")
    d_ones = nc.dram_tensor("ones", [1, BL], tdt, kind="ExternalInput")
    d_y = nc.dram_tensor("y", [BL, 1], f32, kind="ExternalOutput")

    with tile.TileContext(nc) as tc:
        with (
            tc.tile_pool(name="const", bufs=1) as cp,
            tc.tile_pool(name="ps1", bufs=3, space="PSUM") as ps1,
            tc.tile_pool(name="ps2", bufs=3, space="PSUM") as ps2,
            tc.tile_pool(name="psd", bufs=1, space="PSUM") as psd,
            tc.tile_pool(name="gates", bufs=6) as gp,
        ):
            tabs = {}
            for n, d in d_tabs.items():
                tabs[n] = cp.tile(list(d.shape), tdt, tag=n, name=n)
                nc.sync.dma_start(tabs[n][:], d[:])
            wd_sb = cp.tile([U, 1], tdt, tag="wd")
            nc.sync.dma_start(wd_sb[:], d_wd[:])
            bd_sb = cp.tile([BL, 1], f32, tag="bd")
            nc.sync.dma_start(bd_sb[:], d_bd[:])
            # stacked rhs state tiles: rows 0-99 h, row 100 ones, 101-126 oh
            stk1 = []
            stk2 = []
            for k in range(NB):
                s1 = cp.tile([127, BL], tdt, tag=f"stk1_{k}", name=f"stk1_{k}")
                s2 = cp.tile([127, BL], tdt, tag=f"stk2_{k}", name=f"stk2_{k}")
                nc.vector.memset(s1[:].bitcast(f32), 0.0)
                nc.vector.memset(s2[:].bitcast(f32), 0.0)
                nc.sync.dma_start(s1[100:101, :], d_ones[:])
                nc.sync.dma_start(s2[100:101, :], d_ones[:])
                stk1.append(s1)
                stk2.append(s2)
            out_sb = cp.tile([BL, 1], f32, tag="out")

            def gate_block(P, hprev_state, h_out):
                """P: psum [100, 4*BL] blocks zbar|r|xh|rech -> h' into h_out."""
                s = gp.tile([U, 2 * BL], f32, tag="s")
                nc.scalar.activation(s[:], P[:, 0 : 2 * BL], AFT.Sigmoid)
                q = gp.tile([U, BL], f32, tag="q")
                nc.vector.tensor_tensor(
                    q[:], s[:, BL : 2 * BL], P[:, 3 * BL : 4 * BL], op=Alu.mult
                )
                p = gp.tile([U, BL], f32, tag="p")
                nc.vector.tensor_tensor(
                    p[:], q[:], P[:, 2 * BL : 3 * BL], op=Alu.add
                )
                hh = gp.tile([U, BL], f32, tag="hh")
                nc.scalar.activation(hh[:], p[:], AFT.Tanh)
                v = gp.tile([U, BL], f32, tag="v")
                nc.vector.scalar_tensor_tensor(
                    v[:], s[:, 0:BL], 1.0, hprev_state,
                    op0=Alu.subtract, op1=Alu.mult,
                )
                u = gp.tile([U, BL], f32, tag="u")
                nc.gpsimd.tensor_tensor(u[:], s[:, 0:BL], hh[:], op=Alu.mult)
                nc.vector.tensor_tensor(h_out, u[:], v[:], op=Alu.subtract)

            mm = nc.tensor.matmul
            for t in range(nsteps):
                k_t = t % NB
                k_n = (t + 1) % NB
                s1_t, s1_n = stk1[k_t], stk1[k_n]
                s2_t, s2_n = stk2[k_t], stk2[k_n]

                # one-hot for step t+1 into the next stacked tiles (off-chain)
                if t + 1 < nsteps:
                    nc.sync.dma_start(s1_n[101:127, :], d_oh[t + 1])
                    nc.sync.dma_start(s2_n[101:127, :], d_oh[t + 1])
                if t == 0:
                    nc.sync.dma_start(s1_t[101:127, :], d_oh[0])
                    nc.sync.dma_start(s2_t[101:127, :], d_oh[0])

                # ---- layer 2 recurrent part (only needs h2_{t-1}) ----
                P2 = ps2.tile([U, 4 * BL], f32, tag="P2")
                mm(P2[:, 0:BL], mmcast(tabs["t2z"][:]), mmcast(s2_t[:]),
                   start=True, stop=False, skip_group_check=True)
                mm(P2[:, BL : 2 * BL], mmcast(tabs["t2r"][:]), mmcast(s2_t[:]),
                   start=False, stop=False, skip_group_check=True)
                mm(P2[:, 3 * BL : 4 * BL], mmcast(tabs["t2c"][:]), mmcast(s2_t[:]),
                   start=False, stop=False, skip_group_check=True)

                # ---- layer 1, step t ----
                P1 = ps1.tile([U, 4 * BL], f32, tag="P1")
                mm(P1[:, 0:BL], mmcast(tabs["t1z"][:]), mmcast(s1_t[:]),
                   start=True, stop=False, skip_group_check=True)
                mm(P1[:, BL : 2 * BL], mmcast(tabs["t1r"][:]), mmcast(s1_t[:]),
                   start=False, stop=False, skip_group_check=True)
                mm(P1[:, 2 * BL : 3 * BL], mmcast(tabs["t1x"][:]), mmcast(s1_t[:]),
                   start=False, stop=False, skip_group_check=True)
                mm(P1[:, 3 * BL : 4 * BL], mmcast(tabs["t1c"][:]), mmcast(s1_t[:]),
                   start=False, stop=True, skip_group_check=True)
                gate_block(P1, s1_t[0:100, :], s1_n[0:100, :])

                # ---- layer 2 input part (needs h1_t = s1_n rows 0-100) ----
                mm(P2[:, 0:BL], mmcast(tabs["w2z"][:]), mmcast(s1_n[0:101, :]),
                   start=False, stop=False, skip_group_check=True)
                mm(P2[:, BL : 2 * BL], mmcast(tabs["w2r"][:]), mmcast(s1_n[0:101, :]),
                   start=False, stop=False, skip_group_check=True)
                mm(P2[:, 2 * BL : 3 * BL], mmcast(tabs["w2x"][:]), mmcast(s1_n[0:101, :]),
                   start=False, stop=True, skip_group_check=True)
                gate_block(P2, s2_t[0:100, :], s2_n[0:100, :])

            # ---- dense head ----
            s2_fin = stk2[nsteps % NB]
            pd = psd.tile([BL, 1], f32, tag="pd")
            mm(pd[:], mmcast(s2_fin[0:100, :]), mmcast(wd_sb[:]),
               start=True, stop=True)
            nc.scalar.activation(out_sb[:], pd[:], AFT.Identity,
                                 bias=bd_sb[:, 0:1])
            nc.sync.dma_start(d_y[:], out_sb[:])

    return nc


_CACHE = {}


def kernel(tokens, emb, W1, U1, b1, W2, U2, b2, Wd, bd):
    _install_compile_patch()
    from concourse.bass_utils import run_bass_kernel_spmd

    tokens = np.asarray(tokens)
    np_dt = np.float32 if MM_DTYPE in ("f32", "f32r") else __import__("ml_dtypes").bfloat16
    tables = _build_tables(
        np.asarray(emb, np.float32), np.asarray(W1, np.float32),
        np.asarray(U1, np.float32), np.asarray(b1, np.float32),
        np.asarray(W2, np.float32), np.asarray(U2, np.float32),
        np.asarray(b2, np.float32), np_dt=np_dt,
    )
    wd = np.ascontiguousarray(np.asarray(Wd, np.float32).reshape(U, 1).astype(np_dt))
    bdv = np.full((BL, 1), np.float32(np.asarray(bd).reshape(-1)[0]), np.float32)
    ones = np.ones((1, BL), np_dt)

    if "nc" not in _CACHE:
        _CACHE["nc"] = _build_program()
    nc = _CACHE["nc"]

    in_maps = []
    for c in range(NCORES):
        m = {"oh": _build_onehot(tokens[c * BL : (c + 1) * BL], np_dt)}
        m.update(tables)
        m["wd"] = wd
        m["bd"] = bdv
        m["ones"] = ones
        in_maps.append(m)

    res = run_bass_kernel_spmd(nc, in_maps, core_ids=list(range(NCORES)))
    _CACHE["last_result"] = res
    out = np.concatenate([res.results[c]["y"] for c in range(NCORES)], axis=0)
    return out.astype(np.float32)


if __name__ == "__main__":
    tok = np.random.randint(0, V, (B, T), dtype=np.int32)
    rng = np.random.default_rng(0)
    args = dict(
        tokens=tok,
        emb=rng.normal(size=(V, E)).astype(np.float32) * 0.05,
        W1=rng.normal(size=(E, 3 * U)).astype(np.float32) * 0.08,
        U1=rng.normal(size=(U, 3 * U)).astype(np.float32) * 0.1,
        b1=np.zeros((2, 3 * U), np.float32),
        W2=rng.normal(size=(U, 3 * U)).astype(np.float32) * 0.1,
        U2=rng.normal(size=(U, 3 * U)).astype(np.float32) * 0.1,
        b2=np.zeros((2, 3 * U), np.float32),
        Wd=rng.normal(size=(U, 1)).astype(np.float32) * 0.1,
        bd=np.zeros((1,), np.float32),
    )
    print(kernel(**args)[:4])
